# revision 1
# baseline (speedup 1.0000x reference)
"""Trainium2 Bass kernel for nn_Attention_45148696216391.

Multi-head attention with QK L2-norm (qk-norm) + learned per-head scales:
  q = x @ Wq.T ; k = x @ Wk.T ; v = x @ Wv.T       (per head, dh=64)
  q = l2norm(q) * q_scale ; k = l2norm(k) * k_scale
  out = softmax(q k^T / sqrt(dh)) @ v ; out = out @ Wo.T + bo

Sharding (8 cores): data parallel over batch b (2) x tensor parallel over
heads (16 heads -> 4 per core).  Each core computes, for its (b, head-group):
    P_out^T = Wo_s^T @ O^T   in (d, n) layout  -- a PARTIAL sum over e-dims.
Host reduces the 4 head-group partials per batch, transposes, adds bo.

Per-core dataflow (everything transposed, d/e on partitions; bf16 matmul
operands, fp32 PSUM accumulation):
  xt (1024, 2048) = x[b].T streamed in [128, 512] tiles.
  Q^T/K^T per (head, i512-block) in [128, 512] tiles, rows 64-127 ZERO:
  half-array matmuls don't register as activity on the PE HAM clock gate
  (the PE sticks at 1.2 GHz), so everything is padded to K=128/M=128.
  q_scale/sqrt(dh), k_scale are folded into the weights host-side; the
  l2-norm 'undoes' them via a 1/s^2-valued reduction mask (ss = mask.T@q'^2).
  1/||q|| via reciprocal_approx_fast (exact reciprocal costs ~6.5 cyc/elem),
  replicated across partitions by a bounce through DRAM (engines cannot
  partition-broadcast on SBUF).
  V natural per j-chunk in [128, 4*128] tiles: per head 64 V cols, a ones
  col (makes the PV matmul also emit the softmax denominator Z), zero pad.
  scores S^T[j, i] in psum pairs [128, 1024] (two j-tiles) -> exp on ACT
  (no max subtraction: q,k unit vectors so |s| <= q_scale*k_scale/8) ->
  PV accumulates O^T[dh + Z + pad, i] over 16 j-tiles.
  epilogue: O^T copied to SBUF (frees the psum bank fast), 1/Z in f32
  replicated via DRAM bounce, one multiply into OC.
  out-proj per i512: psum[d-tile, i] = sum_ec WoT[ec] @ O^T[ec] -> DRAM.

Emission order engineered for overlap: K-ec0/Q-ec0/V first, then the
chunk-0 attention stream starts while the ec1 projections hide inside its
exp-bound window; out-proj is staggered one block behind attention.
All tiles are per-(head, i512)/per-j-chunk so the Tile scheduler sees
independent dataflow with no false WAR/WAW serialization.
"""

import os
import sys

sys.path.insert(0, "/opt/trn_rl_repo")

import numpy as np

import concourse.bacc as bacc
import concourse.mybir as mybir
import concourse.tile as tile

B, N, DIM = 2, 2048, 1024
H, DH = 16, 64
E = 256            # inner dims per core (4 heads x 64)
NC = 8             # cores
HPC = 4            # heads per core
I512 = 512         # i-tile
NI = N // I512     # 4 i-blocks
NDC = DIM // 128   # 8 d-chunks
NJT = N // 128     # 16 j-tiles

f32 = mybir.dt.float32
f32r = mybir.dt.float32r
bf16 = mybir.dt.bfloat16

# matmul operand dtype: bf16 (full PE rate, FWL, HAM warms) | f32r | f32
MM_DT = os.environ.get("KMM_DT", "bf16")
MMD = {"bf16": bf16, "f32r": f32r, "f32": f32}[MM_DT]


def build_nc():
    nc = bacc.Bacc("TRN2", target_bir_lowering=False, debug=False)

    xt = nc.dram_tensor("xt", [DIM, N], MMD, kind="ExternalInput").ap()
    wqt = nc.dram_tensor("wqt", [DIM, E], MMD, kind="ExternalInput").ap()
    wkt = nc.dram_tensor("wkt", [DIM, E], MMD, kind="ExternalInput").ap()
    wvt = nc.dram_tensor("wvt", [DIM, E], MMD, kind="ExternalInput").ap()
    wot = nc.dram_tensor("wot", [E, DIM], MMD, kind="ExternalInput").ap()
    hmk = nc.dram_tensor("hmk", [128, 66], MMD, kind="ExternalInput").ap()
    nmq = nc.dram_tensor("nmq", [128, 2, 2], MMD, kind="ExternalInput").ap()
    nmk = nc.dram_tensor("nmk", [128, 2, 2], MMD, kind="ExternalInput").ap()
    out = nc.dram_tensor("out", [DIM, N], f32, kind="ExternalOutput").ap()

    with tile.TileContext(nc) as tc:
        with (
            tc.tile_pool(name="wpool", bufs=1) as wpool,
            tc.tile_pool(name="big", bufs=1) as big,
            tc.tile_pool(name="xts", bufs=4) as xts,
            tc.tile_pool(name="sqp", bufs=3) as sqp,
            tc.tile_pool(name="nsp", bufs=4) as nsp,
            tc.tile_pool(name="ptp", bufs=8) as ptp,
            tc.tile_pool(name="obp", bufs=3) as obp,
            tc.tile_pool(name="zdp", bufs=6, space="DRAM") as zdp,
            tc.tile_pool(name="pa", bufs=3, space="PSUM") as pa,
            tc.tile_pool(name="po", bufs=2, space="PSUM") as po,
        ):
            # ---- weights + constants in SBUF ----
            WQT = wpool.tile([128, NDC, E], MMD)  # [d_in_chunk, dc, e]
            WKT = wpool.tile([128, NDC, E], MMD)
            WVT = wpool.tile([128, NDC, E], MMD)
            WOT = wpool.tile([128, 2, DIM], MMD)  # [e_in_chunk, ec, d]
            nc.sync.dma_start(WKT[:], wkt.rearrange("(dc p) e -> p dc e", p=128))
            nc.sync.dma_start(WQT[:], wqt.rearrange("(dc p) e -> p dc e", p=128))
            nc.sync.dma_start(WVT[:], wvt.rearrange("(dc p) e -> p dc e", p=128))
            nc.sync.dma_start(WOT[:], wot.rearrange("(ec p) d -> p ec d", p=128))
            HM = wpool.tile([128, 66], MMD)  # cols 0-1: head mask; 2-65: ones
            nc.sync.dma_start(HM[:], hmk)
            # norm-reduction masks with 1/s^2 folded in: ss = mask.T @ q'^2
            # recovers ||q||^2 of the unscaled q (weights carry s)
            NMQ = wpool.tile([128, 2, 2], MMD)
            NMK = wpool.tile([128, 2, 2], MMD)
            nc.sync.dma_start(NMQ[:], nmq)
            nc.sync.dma_start(NMK[:], nmk)

            # ---- per-block persistent tiles (independent dataflow units) ----
            QT = [
                [big.tile([128, I512], MMD, name=f"qt{h}_{i}", tag=f"qt{h}_{i}")
                 for i in range(NI)]
                for h in range(HPC)
            ]
            KT = [
                [big.tile([128, I512], MMD, name=f"kt{h}_{i}", tag=f"kt{h}_{i}")
                 for i in range(NI)]
                for h in range(HPC)
            ]
            OC = [
                [big.tile([128, I512], MMD, name=f"oc{c}_{i}", tag=f"oc{c}_{i}")
                 for i in range(NI)]
                for c in range(2)
            ]
            VA = [
                big.tile([128, HPC * 128], MMD, name=f"va{j}", tag=f"va{j}")
                for j in range(NJT)
            ]
            for h in range(HPC):
                for i in range(NI):
                    nc.gpsimd.memset(QT[h][i][64:128, :], 0.0)
                    nc.gpsimd.memset(KT[h][i][64:128, :], 0.0)
            for j in range(NJT):
                nc.gpsimd.memset(VA[j][:], 0.0)
                nc.vector.tensor_copy(
                    VA[j].rearrange("p (h c) -> p h c", c=128)[:, :, 64:65],
                    HM[:, 2:3].to_broadcast([128, HPC, 1]),
                )

            # ---- projections ----
            # K for all blocks first, then V, then Q just-in-time per
            # attention block: attention(i5=0) becomes runnable after
            # ~60% of projection work and its ACT-bound stretch absorbs
            # the remaining Q projections.
            xtls = []
            for i5 in range(NI):
                isl = slice(i5 * I512, (i5 + 1) * I512)
                xb = xts.tile([128, NDC, I512], MMD, tag="xt", name=f"xb{i5}")
                if i5 == 0:
                    # first block: per-chunk DMAs so the very first matmul
                    # starts after 128KB instead of 1MB
                    for dc in range(NDC):
                        nc.sync.dma_start(
                            xb[:, dc, :], xt[128 * dc : 128 * (dc + 1), isl]
                        )
                else:
                    nc.sync.dma_start(
                        xb[:], xt.rearrange("(dc p) n -> p dc n", p=128)[:, :, isl]
                    )
                xtls.append([xb[:, dc, :] for dc in range(NDC)])

            def qk_proj(i5, ec, WT, NM, DST):
                if True:
                    xtl = xtls[i5]
                    pq = pa.tile([128, I512], f32, tag="A", name="pq")
                    for dc in range(NDC):
                        nc.tensor.matmul(
                            pq[:],
                            WT[:, dc, 128 * ec : 128 * (ec + 1)],
                            xtl[dc][:],
                            start=(dc == 0),
                            stop=(dc == NDC - 1),
                        )
                    # the 1/s^2 descale rides in the reduction mask
                    sq = sqp.tile([128, I512], MMD, tag="sq")
                    nc.scalar.activation(
                        sq[:], pq[:], mybir.ActivationFunctionType.Square
                    )
                    pnn = po.tile([2, I512], f32, tag="po", name="pnn")
                    nc.tensor.matmul(
                        pnn[:], NM[:, ec, :], sq[:], start=True, stop=True
                    )
                    ns = nsp.tile([2, I512], f32, tag="ns")
                    nc.scalar.activation(
                        ns[:], pnn[:], mybir.ActivationFunctionType.Sqrt
                    )
                    rq = nsp.tile([2, I512], f32, tag="rq")
                    nc.vector.reciprocal_approx_fast(rq[:], ns[:])
                    rd = zdp.tile([2, I512], f32, tag="rd")
                    nc.sync.dma_start(rd[:], rq[:])
                    for hh in range(2):
                        h = 2 * ec + hh
                        rr = sqp.tile([64, I512], f32, tag="rr")
                        nc.sync.dma_start(
                            rr[:], rd[hh : hh + 1, :].to_broadcast([64, I512])
                        )
                        nc.vector.tensor_tensor(
                            DST[h][i5][0:64, :],
                            pq[64 * hh : 64 * hh + 64, :],
                            rr[:],
                            mybir.AluOpType.mult,
                        )

            for i5 in range(NI):
                qk_proj(i5, 0, WKT, NMK, KT)
            for i5 in range(NI):
                qk_proj(i5, 0, WQT, NMQ, QT)

            # V: natural layout, x^T tiles stationary
            for nt in range(NJT):
                i5, ntl = divmod(nt, 4)
                pv = pa.tile([128, E], f32, tag="A", name="pv")
                for dc in range(NDC):
                    nc.tensor.matmul(
                        pv[:],
                        xtls[i5][dc][:, 128 * ntl : 128 * (ntl + 1)],
                        WVT[:, dc, :],
                        start=(dc == 0),
                        stop=(dc == NDC - 1),
                    )
                nc.vector.tensor_copy(
                    VA[nt].rearrange("p (h c) -> p h c", c=128)[:, :, 0:64],
                    pv[:].rearrange("p (h c) -> p h c", c=64),
                )

            # ---- attention + staggered output projection ----
            def outproj(i5):
                isl = slice(i5 * I512, (i5 + 1) * I512)
                for dt in range(NDC):
                    pp_o = pa.tile([128, I512], f32, tag="A", name="ppo")
                    for ec in range(2):
                        nc.tensor.matmul(
                            pp_o[:],
                            WOT[:, ec, 128 * dt : 128 * (dt + 1)],
                            OC[ec][i5][:],
                            start=(ec == 0),
                            stop=(ec == 1),
                        )
                    ob = obp.tile([128, I512], f32, tag="ob")
                    nc.vector.tensor_copy(ob[:], pp_o[:])
                    nc.sync.dma_start(out[128 * dt : 128 * (dt + 1), isl], ob[:])

            def att_block(i5, c):
                isl = slice(i5 * I512, (i5 + 1) * I512)
                if True:
                    pos = [
                        po.tile([128, I512], f32, tag="po", name=f"po{_d}")
                        for _d in range(2)
                    ]
                    for jp in range(NJT // 2):  # pairs of j-tiles
                        pscs = [
                            pa.tile([128, 1024], f32, tag="A", name=f"psc{_d}")
                            for _d in range(2)
                        ]
                        for d in range(2):
                            h = 2 * c + d
                            for u in range(2):
                                jt = 2 * jp + u
                                nc.tensor.matmul(
                                    pscs[d][:, 512 * u : 512 * (u + 1)],
                                    KT[h][jt // 4][:, 128 * (jt % 4) : 128 * (jt % 4) + 128],
                                    QT[h][i5][:],
                                    start=True,
                                    stop=True,
                                )
                        pts = []
                        for d in range(2):
                            pt = ptp.tile([128, 1024], MMD, tag="pt")
                            nc.scalar.activation(
                                pt[:], pscs[d][:], mybir.ActivationFunctionType.Exp
                            )
                            pts.append(pt)
                        for d in range(2):
                            h = 2 * c + d
                            for u in range(2):
                                jt = 2 * jp + u
                                nc.tensor.matmul(
                                    pos[d][:],
                                    VA[jt][:, 128 * h : 128 * h + 128],
                                    pts[d][:, 512 * u : 512 * (u + 1)],
                                    start=(jt == 0),
                                    stop=(jt == NJT - 1),
                                )
                    # epilogue: O^T+Z to SBUF (frees the po bank), divide by Z
                    for d in range(2):
                        ot = nsp.tile([65, I512], f32, tag="ot")
                        nc.vector.tensor_copy(ot[:], pos[d][0:65, :])
                        # Z to partition 0: reciprocal_approx_fast (custom DVE
                        # op) misbehaves at nonzero base partitions
                        zrow = nsp.tile([1, I512], f32, tag="zrow")
                        nc.vector.tensor_copy(zrow[:], pos[d][64:65, :])
                        rz = nsp.tile([1, I512], f32, tag="rz")
                        nc.vector.reciprocal_approx_fast(rz[:], zrow[:])
                        zd = zdp.tile([1, I512], f32, tag="zd")
                        nc.sync.dma_start(zd[:], rz[:])
                        rzr = nsp.tile([64, I512], f32, tag="rzr")
                        nc.sync.dma_start(rzr[:], zd[:].to_broadcast([64, I512]))
                        nc.vector.tensor_tensor(
                            OC[c][i5][64 * d : 64 * (d + 1), :],
                            ot[0:64, :],
                            rzr[:],
                            mybir.AluOpType.mult,
                        )

            # chunk-0 attention starts as soon as ec0 projections + V are
            # done; the ec1 projections hide inside its ACT-bound window
            att_block(0, 0)
            for i5 in range(NI):
                qk_proj(i5, 1, WKT, NMK, KT)
            for i5 in range(NI):
                qk_proj(i5, 1, WQT, NMQ, QT)
            for i5 in range(1, NI):
                att_block(i5, 0)
            # c1 stream in reverse i5 order with the out-proj staggered one
            # block back: every out-proj except the last overlaps attention,
            # and only outproj(0) trails the final block
            c1_order = list(range(NI - 1, -1, -1))
            for idx, i5 in enumerate(c1_order):
                att_block(i5, 1)
                if idx > 0:
                    outproj(c1_order[idx - 1])
            outproj(c1_order[-1])

    nc.compile()
    return nc


def make_in_maps(x, Wq, Wk, Wv, Wo, q_scale, k_scale):
    """Shard + lay out the full inputs for the 8 cores."""
    npdt = mybir.dt.np(MMD)
    x = np.asarray(x, dtype=np.float32)
    Wq = np.asarray(Wq, dtype=np.float32)
    Wk = np.asarray(Wk, dtype=np.float32)
    Wv = np.asarray(Wv, dtype=np.float32)
    Wo = np.asarray(Wo, dtype=np.float32)
    qs = np.asarray(q_scale, dtype=np.float32).reshape(H, DH)
    ks = np.asarray(k_scale, dtype=np.float32).reshape(H, DH)

    hmk = np.zeros((128, 66), np.float32)
    hmk[0:64, 0] = 1.0
    hmk[64:128, 1] = 1.0
    hmk[:, 2:66] = 1.0

    xts_ = [np.ascontiguousarray(x[b].T).astype(npdt) for b in range(B)]
    hmk = hmk.astype(npdt)
    in_maps = []
    for core in range(NC):
        b, g = divmod(core, 4)
        esl = slice(E * g, E * (g + 1))
        qsv = qs[HPC * g : HPC * g + HPC].reshape(E) * DH ** -0.5  # (256,)
        ksv = ks[HPC * g : HPC * g + HPC].reshape(E)
        nmq = np.zeros((128, 2, 2), np.float32)
        nmk = np.zeros((128, 2, 2), np.float32)
        for ec in range(2):
            for p in range(128):
                nmq[p, ec, p // 64] = 1.0 / qsv[128 * ec + p] ** 2
                nmk[p, ec, p // 64] = 1.0 / ksv[128 * ec + p] ** 2
        in_maps.append(
            {
                "xt": xts_[b],
                "wqt": np.ascontiguousarray(Wq[esl].T * qsv[None, :]).astype(npdt),
                "wkt": np.ascontiguousarray(Wk[esl].T * ksv[None, :]).astype(npdt),
                "wvt": np.ascontiguousarray(Wv[esl].T).astype(npdt),
                "wot": np.ascontiguousarray(Wo[:, esl].T).astype(npdt),
                "hmk": hmk,
                "nmq": nmq.astype(npdt),
                "nmk": nmk.astype(npdt),
            }
        )
    return in_maps


def gather_output(results, bo):
    """results: list of 8 dicts with 'out' (1024, 2048) partial^T arrays."""
    bo = np.asarray(bo, dtype=np.float32)
    out = np.empty((B, N, DIM), np.float32)
    for b in range(B):
        acc = results[4 * b]["out"].astype(np.float32)
        for g in range(1, 4):
            acc = acc + results[4 * b + g]["out"]
        out[b] = acc.T + bo
    return out


_NC_CACHE = {}


def kernel(x, Wq, Wk, Wv, Wo, bo, q_scale, k_scale):
    from concourse.bass_utils import run_bass_kernel_spmd

    key = MM_DT
    if key not in _NC_CACHE:
        _NC_CACHE[key] = build_nc()
    nc = _NC_CACHE[key]
    in_maps = make_in_maps(x, Wq, Wk, Wv, Wo, q_scale, k_scale)
    res = run_bass_kernel_spmd(nc, in_maps, list(range(NC)))
    return gather_output(res.results, bo)

